# revision 25
# baseline (speedup 1.0000x reference)
"""Trainium2 Bass kernel for MetaGNNNoEdgeAttr (GNN message passing).

Strategy (8 NeuronCores, SPMD):
  - Undirected+self-loop graph; bipartite => edges into half-A nodes have
    src in half B and vice versa.
  - Nodes are packed into (core, block, slot): 4 cores per half, BLOCKS
    blocks of 128 slots per core, degree-balanced so every block has at
    most SUBT*128 incoming directed edges -> fully uniform SPMD schedule.
  - Algebra: kqv/W1 folded on host:  Kp = x @ Wkp, Qp = x @ Wqp, V = x @ Wv
    with Wkp = (Wk/sqrt(hd)) @ blockdiag(W1k) etc.  Then per edge
    a = W2^T relu(Kp[src] + Qp[dst]) and out_i = (sum_e exp(a) * V[src]) /
    (sum_e exp(a)) @ Wout (+x, relu).  Segment max is skipped (|a| ~ 0.07).
  - fp8 tables: Kp/Qp scaled by SK=256 (relu is pos-homogeneous, W2/SK
    absorbs it), V scaled by SV=8 (ones_col=SV makes the softmax ratio
    exact).  Kp and V fused in ONE [SLOTS, 512] fp8 table -> a single fp8
    AllGather (per half-group) and a single 512B-per-edge gather.
  - Phase A pass 1 (per core): Kp|V for partner core's nodes -> fp8 table
    -> group AllGather.  Pass 2 (overlaps the collective): Qp table (fp8,
    local), Vown (fp8 resident), ST=(Kp+Qp)^T (bf16 resident) for own
    nodes; then self-loop logits a_self = W2'.relu(ST) for all blocks.
  - Edge phase per gather batch: one fused KpV gather (edge-major fp8) +
    one Qp gather; z=Kp+Qp (DVE), relu (ACT), a = rowsum(rz*W2rep) (DVE
    mult + per-head strided reduce); exp on ACT; per (subtile, head)
    one-hot(dst)*exp via fused tensor_scalar; aggregation + softmax
    denominator via PE matmuls into PSUM; self-loops via resident
    Vown/exp(a_self) diagonals; Wout per head; per-node normalization via
    ACT scale; relu; +x residual (bf16); batched output stores.
"""

import os
import sys
import math
import numpy as np

for _p in ("/opt/trn_rl_repo", "/root/.axon_site/_ro/trn_rl_repo"):
    if os.path.isdir(_p) and _p not in sys.path:
        sys.path.insert(0, _p)

import ml_dtypes  # noqa: E402

BF16 = ml_dtypes.bfloat16
FP8 = ml_dtypes.float8_e4m3

SK = 256.0  # Kp/Qp fp8 scale (folded into Wkp/Wqp; W2 divided by it)
SV = 8.0    # V fp8 scale (ones_col = SV cancels it in the softmax ratio)

# ---------------------------------------------------------------- host prep


def _pack_half(deg_half, n_bins, cap_edges):
    """Pack nodes (by local id within half) into n_bins bins of <=128 nodes,
    each bin with sum(deg) <= cap_edges. Returns [n_bins] lists of local ids."""
    nh = deg_half.shape[0]
    order = np.argsort(-deg_half, kind="stable")
    per_bin = (nh + n_bins - 1) // n_bins
    assert per_bin <= 128
    bins = [[] for _ in range(n_bins)]
    loads = np.zeros(n_bins, dtype=np.int64)
    # deal degree-sorted nodes in rounds: heaviest of each round to the
    # currently lightest bins
    for r in range(per_bin):
        chunk = order[r * n_bins : (r + 1) * n_bins]
        target = np.argsort(loads, kind="stable")
        for k, nd in enumerate(chunk):
            b = int(target[k])
            bins[b].append(int(nd))
            loads[b] += deg_half[nd]
    # fixup: move low-degree nodes from overloaded to underloaded bins
    sizes = np.array([len(x) for x in bins])
    for _ in range(20000):
        hi = int(np.argmax(loads))
        if loads[hi] <= cap_edges:
            break
        lo = int(np.argmin(loads + (sizes >= 128) * 10**9))
        bl = bins[hi]
        j = int(np.argmin(deg_half[bl]))
        nd = bl.pop(j)
        bins[lo].append(nd)
        loads[hi] -= deg_half[nd]
        loads[lo] += deg_half[nd]
        sizes[hi] -= 1
        sizes[lo] += 1
    assert loads.max() <= cap_edges, (loads.max(), cap_edges)
    assert max(len(b) for b in bins) <= 128
    return bins


def prep(x, edge_index, Wkqv, bkqv, W1, b1, W2, b2, Wout, bout, n_cores=8):
    """All host-side preprocessing. Returns (meta, per_core_inputs, const_inputs,
    scatter info for output assembly)."""
    N, EMB = x.shape
    H, hd = 2, EMB // 2
    HALF = N // 2
    GROUP = n_cores // 2

    ei = np.asarray(edge_index).astype(np.int64)
    src = np.concatenate([ei[0], ei[1]])
    dst = np.concatenate([ei[1], ei[0]])
    assert src.min() >= 0 and src.max() < N

    deg = np.bincount(dst, minlength=N)

    # ---- choose BLOCKS / SUBT: minimize total subtiles T = BLOCKS * SUBT
    B0 = max(1, (HALF + GROUP * 128 - 1) // (GROUP * 128))
    best = None
    for BLOCKS in range(B0, B0 + 4):
        n_bins = GROUP * BLOCKS
        load = max(deg[:HALF].sum(), deg[HALF:].sum()) / n_bins
        SUBT = max(1, int(math.ceil(load / 128)))
        for _try in range(3):
            cap = SUBT * 128
            try:
                bins_a = _pack_half(deg[:HALF], n_bins, cap)
                bins_b = _pack_half(deg[HALF:], n_bins, cap)
            except AssertionError:
                SUBT += 1
                continue
            if best is None or BLOCKS * SUBT < best[0] * best[1]:
                best = (BLOCKS, SUBT, bins_a, bins_b)
            break
    if best is None:
        raise RuntimeError("packing failed")
    BLOCKS, SUBT, bins_a, bins_b = best
    n_bins = GROUP * BLOCKS
    SLOTS = BLOCKS * 128
    T = BLOCKS * SUBT  # subtiles per core

    # ---- node -> (core, block, slot); core 0..GROUP-1 own half A
    core_of = np.full(N, -1, np.int32)
    block_of = np.full(N, -1, np.int32)
    slot_of = np.full(N, -1, np.int32)
    node_of = np.full((n_cores, SLOTS), -1, np.int64)
    for half, bins in ((0, bins_a), (1, bins_b)):
        for i, bl in enumerate(bins):
            c = half * GROUP + i % GROUP
            b = i // GROUP
            for s, nd_local in enumerate(sorted(bl)):
                nd = nd_local + half * HALF
                core_of[nd] = c
                block_of[nd] = b
                slot_of[nd] = s
                node_of[c, b * 128 + s] = nd
    assert (core_of >= 0).all()

    # opposite-half row of a node (position in the group AllGather output):
    # contribution order within each group AG is group position 0..GROUP-1.
    H1 = SLOTS
    own_row = block_of * 128 + slot_of
    gp = (core_of % GROUP).astype(np.int64)
    opp_row = gp * SLOTS + own_row

    # ---- per-core edge slot assignment
    # gather batch = BPB whole blocks (~2048 edge slots)
    BPB = max(1, 2048 // (SUBT * 128))
    GB = BPB * SUBT * 128  # edge slots per gather batch
    NB = (BLOCKS + BPB - 1) // BPB
    per_core = []
    ecore = core_of[dst]
    for c in range(n_cores):
        m = ecore == c
        es, ed = src[m], dst[m]
        eb = block_of[ed]
        order = np.argsort(eb, kind="stable")
        es, ed, eb = es[order], ed[order], eb[order]
        counts = np.bincount(eb, minlength=BLOCKS)
        assert counts.max() <= SUBT * 128
        kp_idx = np.zeros(T * 128, np.int16)
        qp_idx = np.zeros(T * 128, np.int16)
        dstrel = np.full(T * 128, -1.0, np.float32)
        pos = 0
        for b in range(BLOCKS):
            n_b = counts[b]
            sl = slice(b * SUBT * 128, b * SUBT * 128 + n_b)
            seg = slice(pos, pos + n_b)
            kp_idx[sl] = opp_row[es[seg]].astype(np.int16)
            # qp index is LOCAL to the gather batch's block range so the
            # per-batch gather only depends on those blocks' table writes
            g = b // BPB
            qp_idx[sl] = (own_row[ed[seg]] - g * BPB * 128).astype(np.int16)
            dstrel[sl] = slot_of[ed[seg]].astype(np.float32)
            pos += n_b
        assert opp_row[es].max() < 32768 and GROUP * SLOTS < 32768
        assert qp_idx.min() >= 0 and qp_idx.max() < BPB * 128

        # wrap indices: unwrapped[i] = arr[i%16, i//16], replicated to 128 parts
        def wrap(a):
            out = np.zeros((NB, 128, GB // 16), np.int16)
            for g in range(NB):
                seg = a[g * GB : (g + 1) * GB]
                w = np.zeros((16, GB // 16), np.int16)
                n = len(seg)
                ii = np.arange(n)
                w[ii % 16, ii // 16] = seg
                out[g] = np.tile(w, (8, 1))
            return out

        xo = np.zeros((SLOTS, EMB), np.float32)
        vmask = node_of[c] >= 0
        xo[vmask] = x[node_of[c][vmask]]
        p = (c + GROUP) % n_cores  # partner core (opposite half)
        xp = np.zeros((SLOTS, EMB), np.float32)
        vmaskp = node_of[p] >= 0
        xp[vmaskp] = x[node_of[p][vmaskp]]

        def xT(a):  # [SLOTS, EMB] -> [128, EMB//128, SLOTS] fp8
            return np.ascontiguousarray(
                a.reshape(SLOTS, EMB // 128, 128).transpose(2, 1, 0)
            ).astype(FP8)

        per_core.append(
            dict(
                xTp=xT(xp),
                xTo=xT(xo),
                x_own=np.ascontiguousarray(
                    xo.reshape(BLOCKS, 128, EMB).transpose(1, 0, 2)
                ).astype(BF16),
                kp_ix=wrap(kp_idx),
                qp_ix=wrap(qp_idx),
                dstrel=np.ascontiguousarray(dstrel.reshape(T, 128).T),
            )
        )

    # ---- folded weights
    W1k, W1q = W1[:hd].astype(np.float64), W1[hd:].astype(np.float64)
    bd = lambda A: np.block(
        [[A, np.zeros_like(A)], [np.zeros_like(A), A]]
    )  # [256,256]
    Wq = Wkqv[:, :EMB].astype(np.float64)
    Wk = Wkqv[:, EMB : 2 * EMB].astype(np.float64) / math.sqrt(hd)
    Wv = Wkqv[:, 2 * EMB :].astype(np.float64)
    Wkp = (Wk @ bd(W1k)) * SK
    Wqp = (Wq @ bd(W1q)) * SK
    Wkpq = Wkp + Wqp
    Wvs = Wv * SV

    if not (
        np.all(bkqv == 0) and np.all(b1 == 0) and np.all(bout == 0)
    ):
        raise NotImplementedError("nonzero bkqv/b1/bout not supported")

    # SBUF layout [128, 2, F]: [p, c, e] = W[c*128+p, e]
    def chunk(W, F):
        return np.ascontiguousarray(
            W.astype(np.float32).reshape(2, 128, F).transpose(1, 0, 2)
        ).astype(BF16)

    Wkv = np.concatenate([Wkp, Wvs], axis=1)   # [256, 512] -> kp | v
    Wqv = np.concatenate([Wqp, Wvs], axis=1)   # [256, 512] -> qp | vown
    W2s = (W2[:hd].astype(np.float64) / SK).astype(np.float32)  # [128, 1]
    CW = GB // 128  # subtile-columns per full gather batch
    w2row = np.tile(W2s.reshape(-1), 2)  # [256]: W2/SK for both heads
    w2rep = np.ascontiguousarray(
        np.broadcast_to(w2row, (128, CW, 256))
    ).astype(BF16)
    consts = dict(
        Wkv_c=chunk(Wkv, 512),
        Wqv_c=chunk(Wqv, 512),
        Wkpq_c=chunk(Wkpq, EMB),
        Wout_c=chunk(Wout.astype(np.float64), EMB),
        W2_c=np.ascontiguousarray(W2s.astype(BF16)),  # [128,1]
        W2rep_c=w2rep,
        iota_bc=np.tile(np.arange(128, dtype=BF16)[None, :], (128, 1)),
        iota_col=np.arange(128, dtype=np.float32)[:, None],
        ones_col=np.full((128, 1), SV, BF16),
    )
    meta = dict(
        N=N,
        EMB=EMB,
        HALF=HALF,
        GROUP=GROUP,
        BLOCKS=BLOCKS,
        SLOTS=SLOTS,
        SUBT=SUBT,
        T=T,
        GB=GB,
        NB=NB,
        BPB=BPB,
        CW=CW,
        H1=H1,
        n_cores=n_cores,
        b2=float(np.asarray(b2).reshape(-1)[0]),
    )
    return meta, per_core, consts, node_of


# ------------------------------------------------------------- bass program


def build_program(meta, repeats=1):
    import concourse.bass as bass
    import concourse.tile as tile
    from concourse import bacc, mybir

    AF = mybir.ActivationFunctionType
    ALU = mybir.AluOpType
    BF = mybir.dt.bfloat16
    F32 = mybir.dt.float32
    F8 = mybir.dt.float8e4
    I16 = mybir.dt.int16

    EMB = meta["EMB"]
    BLOCKS, SLOTS, SUBT, T = (
        meta["BLOCKS"],
        meta["SLOTS"],
        meta["SUBT"],
        meta["T"],
    )
    GB, NB, GROUP, BPB, CW = (
        meta["GB"], meta["NB"], meta["GROUP"], meta["BPB"], meta["CW"],
    )
    H1 = meta["H1"]
    n_cores = meta["n_cores"]
    b2f = meta["b2"]
    NB4 = (BLOCKS + 3) // 4  # phase-A 4-block chunks
    AG1_C4 = H1 // 512  # pass-1 chunks contributing to AllGather chunk 1

    nc = bacc.Bacc(
        "TRN2", target_bir_lowering=False, debug=False, num_devices=n_cores
    )

    dram = lambda name, shape, dt: nc.dram_tensor(
        name, shape, dt, kind="ExternalInput"
    )
    xTp_d = dram("xTp", [128, 2, SLOTS], F8)
    xTo_d = dram("xTo", [128, 2, SLOTS], F8)
    x_own_d = dram("x_own", [128, BLOCKS, EMB], BF)
    kp_ix_d = dram("kp_ix", [NB, 128, GB // 16], I16)
    qp_ix_d = dram("qp_ix", [NB, 128, GB // 16], I16)
    dstrel_d = dram("dstrel", [128, T], F32)
    Wkv_d = dram("Wkv_c", [128, 2, 512], BF)
    Wqv_d = dram("Wqv_c", [128, 2, 512], BF)
    Wkpq_d = dram("Wkpq_c", [128, 2, EMB], BF)
    Wout_d = dram("Wout_c", [128, 2, EMB], BF)
    W2_d = dram("W2_c", [128, 1], BF)
    W2rep_d = dram("W2rep_c", [128, CW, 256], BF)
    iota_bc_d = dram("iota_bc", [128, 128], BF)
    iota_col_d = dram("iota_col", [128, 1], F32)
    ones_col_d = dram("ones_col", [128, 1], BF)
    out_d = nc.dram_tensor("out", [128, BLOCKS, EMB], BF, kind="ExternalOutput")

    groups2 = [
        list(range(GROUP)),
        list(range(GROUP, n_cores)),
    ]

    with tile.TileContext(nc) as tc:
        # internal DRAM (collective buffers + local qp table)
        _frees = []

        def _dram_tile(shape, name, addr_space="Local"):
            t, free = tc.tile(
                shape,
                F8,
                space=bass.MemorySpace.DRAM,
                addr_space=addr_space,
                name=name,
            )
            _frees.append(free)
            return t

        kpv_agin = _dram_tile([SLOTS, 512], "kpv_agin")
        qp_tbl = _dram_tile([SLOTS, EMB], "qp_tbl")
        kpv_opp = _dram_tile([GROUP * SLOTS, 512], "kpv_opp", "Shared")

        with tc.tile_pool(name="res", bufs=1) as res:
            ST_res = res.tile([128, 2, SLOTS], BF)
            Vown_res = res.tile([128, BLOCKS, EMB], F8)
            x_res = res.tile([128, BLOCKS, EMB], BF)
            exS_res = res.tile([128, BLOCKS, 2], F32)
            dstrel_sb = res.tile([128, T], F32)
            iota_bc = res.tile([128, 128], BF)
            iota_col = res.tile([128, 1], F32)
            ones_col = res.tile([128, 1], BF)
            W2_sb = res.tile([128, 1], BF)
            W2rep_sb = res.tile([128, CW, 256], BF)
            Wkv_sb = res.tile([128, 2, 512], BF)
            Wqv_sb = res.tile([128, 2, 512], BF)
            Wkpq_sb = res.tile([128, 2, EMB], BF)
            Wout_sb = res.tile([128, 2, EMB], BF)

            def _body():
                nc.sync.dma_start(dstrel_sb[:], dstrel_d[:])
                nc.sync.dma_start(iota_bc[:], iota_bc_d[:])
                nc.sync.dma_start(iota_col[:], iota_col_d[:])
                nc.sync.dma_start(ones_col[:], ones_col_d[:])
                nc.sync.dma_start(W2_sb[:], W2_d[:])
                nc.sync.dma_start(W2rep_sb[:], W2rep_d[:])
                nc.sync.dma_start(x_res[:], x_own_d[:])
                for w_sb, w_d in (
                    (Wkv_sb, Wkv_d),
                    (Wqv_sb, Wqv_d),
                    (Wkpq_sb, Wkpq_d),
                    (Wout_sb, Wout_d),
                ):
                    nc.sync.dma_start(w_sb[:], w_d[:])

                # ---------- phase A pass 1: partner Kp|V table -> AllGather
                with (
                    tc.tile_pool(name="pA", bufs=3) as pA,
                    tc.tile_pool(name="psA", bufs=2, space="PSUM") as psA,
                ):
                    for c4 in range(NB4):
                        nb4 = min(BLOCKS - c4 * 4, 4)
                        w = nb4 * 128
                        sl = slice(c4 * 512, c4 * 512 + w)
                        xtp = pA.tile([128, 2, 512], F8, tag="xtp")
                        nc.sync.dma_start(xtp[:, :, :w], xTp_d[:, :, sl])
                        st1 = pA.tile([128, 4, 512], F8, tag="st1")
                        for bi in range(nb4):
                            bsl = slice(bi * 128, (bi + 1) * 128)
                            ps1 = psA.tile([128, 512], F32, tag="ps1")
                            nc.tensor.matmul(
                                ps1[:], xtp[:, 0, bsl], Wkv_sb[:, 0, :],
                                start=True, stop=False,
                            )
                            nc.tensor.matmul(
                                ps1[:], xtp[:, 1, bsl], Wkv_sb[:, 1, :],
                                start=False, stop=True,
                            )
                            nc.vector.tensor_copy(st1[:, bi, :], ps1[:])
                        nc.sync.dma_start(
                            kpv_agin[sl, :].rearrange("(b p) f -> p b f", p=128),
                            st1[:, :nb4, :],
                        )
                nc.gpsimd.collective_compute(
                    "AllGather",
                    ALU.bypass,
                    replica_groups=groups2,
                    ins=[kpv_agin[:]],
                    outs=[kpv_opp[:]],
                )

                # ---------- phase A pass 2 (overlaps collective):
                # own Qp table, Vown, ST; then self-loop logits
                with (
                    tc.tile_pool(name="pB", bufs=3) as pB,
                    tc.tile_pool(name="psB", bufs=2, space="PSUM") as psB,
                    tc.tile_pool(name="psS", bufs=1, space="PSUM") as psS,
                ):
                    aself = psS.tile([128, BLOCKS, 2], F32)
                    for c4 in range(NB4):
                        nb4 = min(BLOCKS - c4 * 4, 4)
                        w = nb4 * 128
                        sl = slice(c4 * 512, c4 * 512 + w)
                        xto = pB.tile([128, 2, 512], F8, tag="xto")
                        nc.sync.dma_start(xto[:, :, :w], xTo_d[:, :, sl])
                        st2 = pB.tile([128, 4, 256], F8, tag="st2")
                        for bi in range(nb4):
                            b = c4 * 4 + bi
                            bsl = slice(bi * 128, (bi + 1) * 128)
                            gsl = slice(b * 128, (b + 1) * 128)
                            ps2 = psB.tile([128, 512], F32, tag="ps2")
                            nc.tensor.matmul(
                                ps2[:], xto[:, 0, bsl], Wqv_sb[:, 0, :],
                                start=True, stop=False,
                            )
                            nc.tensor.matmul(
                                ps2[:], xto[:, 1, bsl], Wqv_sb[:, 1, :],
                                start=False, stop=True,
                            )
                            ps3 = psB.tile([128, 2, 128], F32, tag="ps3")
                            for c in range(2):
                                for f in range(2):
                                    nc.tensor.matmul(
                                        ps3[:, f, :],
                                        Wkpq_sb[:, c, f * 128 : (f + 1) * 128],
                                        xto[:, c, bsl],
                                        start=(c == 0 and f == 0),
                                        stop=(c == 1 and f == 1),
                                    )
                            nc.vector.tensor_copy(st2[:, bi, :], ps2[:, 0:256])
                            nc.scalar.activation(
                                Vown_res[:, b, :], ps2[:, 256:512], AF.Copy
                            )
                            nc.scalar.activation(
                                ST_res[:, :, gsl], ps3[:], AF.Copy
                            )
                            # self-loop logits for this block
                            rst = pB.tile([128, 2, 128], BF, tag="rst")
                            nc.vector.tensor_scalar_max(
                                rst[:], ST_res[:, :, gsl], 0.0
                            )
                            for h in range(2):
                                nc.tensor.matmul(
                                    aself[:, b, h : h + 1],
                                    rst[:, h, :],
                                    W2_sb[:],
                                    start=True,
                                    stop=True,
                                )
                        nc.sync.dma_start(
                            qp_tbl[sl, :].rearrange("(b p) f -> p b f", p=128),
                            st2[:, :nb4, :],
                        )
                    nc.scalar.activation(exS_res[:], aself[:], AF.Exp, bias=b2f)

                # ---------------- edge phase ----------------
                with (
                    tc.tile_pool(name="pG", bufs=2) as pG,
                    tc.tile_pool(name="pW", bufs=2) as pW,
                    tc.tile_pool(name="pE", bufs=3) as pE,
                    tc.tile_pool(name="psE", bufs=2, space="PSUM") as psE,
                ):
                    for g in range(NB):
                        nblk = min(BLOCKS - g * BPB, BPB)
                        C = nblk * SUBT
                        gb = C * 128
                        gb16 = gb // 16
                        kpix = pG.tile([128, GB // 16], I16, tag="kpix")
                        nc.sync.dma_start(kpix[:, :gb16], kp_ix_d[g, :, :gb16])
                        qpix = pG.tile([128, GB // 16], I16, tag="qpix")
                        nc.sync.dma_start(qpix[:, :gb16], qp_ix_d[g, :, :gb16])
                        KpV = pG.tile([128, GB // 128, 512], F8, tag="KpV")
                        nc.gpsimd.dma_gather(
                            KpV[:, :C, :],
                            kpv_opp[:],
                            kpix[:, :gb16],
                            num_idxs=gb,
                            num_idxs_reg=gb,
                            elem_size=512,
                            transpose=False,
                            single_packet=False,
                        )
                        Qp = pG.tile([128, GB // 128, 256], F8, tag="Qp")
                        nc.gpsimd.dma_gather(
                            Qp[:, :C, :],
                            qp_tbl[g * BPB * 128 : g * BPB * 128 + nblk * 128, :],
                            qpix[:, :gb16],
                            num_idxs=gb,
                            num_idxs_reg=gb,
                            elem_size=256,
                            transpose=False,
                            single_packet=False,
                        )
                        z = pW.tile([128, GB // 128, 256], BF, tag="z")
                        nc.vector.tensor_tensor(
                            z[:, :C, :], KpV[:, :C, 0:256], Qp[:, :C, :], ALU.add
                        )
                        rz = pW.tile([128, GB // 128, 256], BF, tag="rz")
                        nc.scalar.activation(rz[:, :C, :], z[:, :C, :], AF.Relu)
                        prod = pW.tile([128, GB // 128, 256], BF, tag="prod")
                        nc.vector.tensor_tensor(
                            prod[:, :C, :], rz[:, :C, :], W2rep_sb[:, :C, :],
                            ALU.mult,
                        )
                        ared = pE.tile([128, GB // 128, 2], F32, tag="ared")
                        for h in range(2):
                            nc.vector.tensor_reduce(
                                ared[:, :C, h : h + 1],
                                prod[:, :C, h * 128 : (h + 1) * 128],
                                axis=mybir.AxisListType.X,
                                op=ALU.add,
                            )
                        ex = pE.tile([128, GB // 128, 2], F32, tag="ex")
                        nc.scalar.activation(
                            ex[:, :C, :], ared[:, :C, :], AF.Exp, bias=b2f
                        )

                        for j in range(nblk):
                            b = g * BPB + j
                            aggs = psE.tile([128, EMB + 2], F32, tag="aggs")
                            first = True
                            for s in range(SUBT):
                                cc = j * SUBT + s
                                t_idx = b * SUBT + s
                                for h in range(2):
                                    oh = pE.tile([128, 128], BF, tag="oh")
                                    nc.vector.tensor_scalar(
                                        oh[:],
                                        iota_bc[:],
                                        dstrel_sb[:, t_idx : t_idx + 1],
                                        ex[:, cc, h : h + 1],
                                        op0=ALU.is_equal,
                                        op1=ALU.mult,
                                    )
                                    nc.tensor.matmul(
                                        aggs[:, h * 128 : (h + 1) * 128],
                                        KpV[:, cc, 256 + h * 128 : 256 + (h + 1) * 128],
                                        oh[:],
                                        start=first,
                                        stop=False,
                                    )
                                    first = False
                                    nc.tensor.matmul(
                                        aggs[:, EMB + h : EMB + h + 1],
                                        oh[:],
                                        ones_col[:],
                                        start=False,
                                        stop=False,
                                    )
                            for h in range(2):
                                dg = pE.tile([128, 128], BF, tag="dg")
                                nc.vector.tensor_scalar(
                                    dg[:],
                                    iota_bc[:],
                                    iota_col[:],
                                    exS_res[:, b, h : h + 1],
                                    op0=ALU.is_equal,
                                    op1=ALU.mult,
                                )
                                nc.tensor.matmul(
                                    aggs[:, h * 128 : (h + 1) * 128],
                                    Vown_res[:, b, h * 128 : (h + 1) * 128],
                                    dg[:],
                                    start=False,
                                    stop=False,
                                )
                                nc.tensor.matmul(
                                    aggs[:, EMB + h : EMB + h + 1],
                                    dg[:],
                                    ones_col[:],
                                    start=False,
                                    stop=(h == 1),
                                )
                            r = pE.tile([128, 2], F32, tag="r")
                            nc.vector.reciprocal(r[:], aggs[:, EMB : EMB + 2])
                            ag0 = pE.tile([128, 128], BF, tag="ag0")
                            nc.vector.tensor_copy(ag0[:], aggs[:, 0:128])
                            ag1 = pE.tile([128, 128], BF, tag="ag1")
                            nc.vector.tensor_copy(ag1[:], aggs[:, 128:256])
                            P = psE.tile([128, 2, EMB], F32, tag="P")
                            nc.tensor.matmul(
                                P[:, 0, :], ag0[:], Wout_sb[:, 0, :],
                                start=True, stop=True,
                            )
                            nc.tensor.matmul(
                                P[:, 1, :], ag1[:], Wout_sb[:, 1, :],
                                start=True, stop=True,
                            )
                            t0 = pE.tile([128, EMB], BF, tag="t0")
                            nc.scalar.activation(
                                t0[:], P[:, 0, :], AF.Copy, scale=r[:, 0:1]
                            )
                            t1 = pE.tile([128, EMB], BF, tag="t1")
                            nc.scalar.activation(
                                t1[:], P[:, 1, :], AF.Copy, scale=r[:, 1:2]
                            )
                            u = pE.tile([128, EMB], BF, tag="u")
                            nc.vector.tensor_tensor(u[:], t0[:], t1[:], ALU.add)
                            rl = pE.tile([128, EMB], BF, tag="rl")
                            nc.scalar.activation(rl[:], u[:], AF.Relu)
                            if j == 0:
                                ost = pE.tile(
                                    [128, BPB, EMB], BF, tag="ost", name="ost"
                                )
                            nc.vector.tensor_tensor(
                                ost[:, j, :], rl[:], x_res[:, b, :], ALU.add
                            )
                        nc.sync.dma_start(
                            out_d[:, g * BPB : g * BPB + nblk, :],
                            ost[:, :nblk, :],
                        )

            for _rep in range(repeats):
                _body()

        for f in _frees:
            f()

    nc.compile()
    return nc


# ------------------------------------------------------------------ driver


def assemble_out(oc, node_of_c, N, EMB, out):
    """Scatter one core's [128, BLOCKS, EMB] output into the full array."""
    blocks = oc.shape[1]
    flat = np.asarray(oc).astype(np.float32).transpose(1, 0, 2).reshape(-1, EMB)
    valid = node_of_c >= 0
    out[node_of_c[valid]] = flat[valid]


def _build_all(inputs, n_cores=8, repeats=1):
    meta, per_core, consts, node_of = prep(n_cores=n_cores, **inputs)
    nc = build_program(meta, repeats=repeats)
    in_maps = []
    for c in range(n_cores):
        m = dict(per_core[c])
        m.update(consts)
        in_maps.append({k: np.ascontiguousarray(v) for k, v in m.items()})
    return meta, nc, in_maps, node_of


def kernel(**inputs):
    import concourse.bass_utils as bass_utils

    n_cores = 8
    meta, nc, in_maps, node_of = _build_all(inputs, n_cores)
    res = bass_utils.run_bass_kernel_spmd(
        nc, in_maps, core_ids=list(range(n_cores))
    )
    N, EMB = inputs["x"].shape
    out = np.zeros((N, EMB), np.float32)
    for c in range(n_cores):
        assemble_out(res.results[c]["out"], node_of[c], N, EMB, out)
    return out


# revision 26
# speedup vs baseline: 1.1568x; 1.1568x over previous
"""Trainium2 Bass kernel for MetaGNNNoEdgeAttr (GNN message passing).

Strategy (8 NeuronCores, SPMD):
  - Undirected+self-loop graph; bipartite => edges into half-A nodes have
    src in half B and vice versa.
  - Nodes are packed into (core, block, slot): 4 cores per half, BLOCKS
    blocks of 128 slots per core, degree-balanced so every block has at
    most SUBT*128 incoming directed edges -> fully uniform SPMD schedule.
  - Algebra: kqv/W1 folded on host:  Kp = x @ Wkp, Qp = x @ Wqp, V = x @ Wv
    with Wkp = (Wk/sqrt(hd)) @ blockdiag(W1k) etc.  Then per edge
    a = W2^T relu(Kp[src] + Qp[dst]) and out_i = (sum_e exp(a) * V[src]) /
    (sum_e exp(a)) @ Wout (+x, relu).  Segment max is skipped (|a| ~ 0.07).
  - fp8 tables: Kp/Qp scaled by SK=256 (relu is pos-homogeneous, W2/SK
    absorbs it), V scaled by SV=8 (ones_col=SV makes the softmax ratio
    exact).  Kp and V fused in ONE [SLOTS, 512] fp8 table -> a single fp8
    AllGather (per half-group) and a single 512B-per-edge gather.
  - Phase A pass 1 (per core): Kp|V for partner core's nodes -> fp8 table
    -> group AllGather.  Pass 2 (overlaps the collective): Qp table (fp8,
    local), Vown (fp8 resident), ST=(Kp+Qp)^T (bf16 resident) for own
    nodes; then self-loop logits a_self = W2'.relu(ST) for all blocks.
  - Edge phase per gather batch: one fused KpV gather (edge-major fp8) +
    one Qp gather; z=Kp+Qp (DVE), relu (ACT), a = rowsum(rz*W2rep) (DVE
    mult + per-head strided reduce); exp on ACT; per (subtile, head)
    one-hot(dst)*exp via fused tensor_scalar; aggregation + softmax
    denominator via PE matmuls into PSUM; self-loops via resident
    Vown/exp(a_self) diagonals; Wout per head; per-node normalization via
    ACT scale; relu; +x residual (bf16); batched output stores.
"""

import os
import sys
import math
import numpy as np

for _p in ("/opt/trn_rl_repo", "/root/.axon_site/_ro/trn_rl_repo"):
    if os.path.isdir(_p) and _p not in sys.path:
        sys.path.insert(0, _p)

import ml_dtypes  # noqa: E402

BF16 = ml_dtypes.bfloat16
FP8 = ml_dtypes.float8_e4m3

SK = 256.0  # Kp/Qp fp8 scale (folded into Wkp/Wqp; W2 divided by it)
SV = 8.0    # V fp8 scale (ones_col = SV cancels it in the softmax ratio)

# ---------------------------------------------------------------- host prep


def _pack_half(deg_half, n_bins, cap_edges):
    """Pack nodes (by local id within half) into n_bins bins of <=128 nodes,
    each bin with sum(deg) <= cap_edges. Returns [n_bins] lists of local ids."""
    nh = deg_half.shape[0]
    order = np.argsort(-deg_half, kind="stable")
    per_bin = (nh + n_bins - 1) // n_bins
    assert per_bin <= 128
    bins = [[] for _ in range(n_bins)]
    loads = np.zeros(n_bins, dtype=np.int64)
    # deal degree-sorted nodes in rounds: heaviest of each round to the
    # currently lightest bins
    for r in range(per_bin):
        chunk = order[r * n_bins : (r + 1) * n_bins]
        target = np.argsort(loads, kind="stable")
        for k, nd in enumerate(chunk):
            b = int(target[k])
            bins[b].append(int(nd))
            loads[b] += deg_half[nd]
    # fixup: move low-degree nodes from overloaded to underloaded bins
    sizes = np.array([len(x) for x in bins])
    for _ in range(20000):
        hi = int(np.argmax(loads))
        if loads[hi] <= cap_edges:
            break
        lo = int(np.argmin(loads + (sizes >= 128) * 10**9))
        bl = bins[hi]
        j = int(np.argmin(deg_half[bl]))
        nd = bl.pop(j)
        bins[lo].append(nd)
        loads[hi] -= deg_half[nd]
        loads[lo] += deg_half[nd]
        sizes[hi] -= 1
        sizes[lo] += 1
    assert loads.max() <= cap_edges, (loads.max(), cap_edges)
    assert max(len(b) for b in bins) <= 128
    return bins


def prep(x, edge_index, Wkqv, bkqv, W1, b1, W2, b2, Wout, bout, n_cores=8):
    """All host-side preprocessing. Returns (meta, per_core_inputs, const_inputs,
    scatter info for output assembly)."""
    N, EMB = x.shape
    H, hd = 2, EMB // 2
    HALF = N // 2
    GROUP = n_cores // 2

    ei = np.asarray(edge_index).astype(np.int64)
    src = np.concatenate([ei[0], ei[1]])
    dst = np.concatenate([ei[1], ei[0]])
    assert src.min() >= 0 and src.max() < N

    deg = np.bincount(dst, minlength=N)

    # ---- choose BLOCKS / SUBT: minimize total subtiles T = BLOCKS * SUBT
    B0 = max(1, (HALF + GROUP * 128 - 1) // (GROUP * 128))
    best = None
    for BLOCKS in range(B0, B0 + 4):
        n_bins = GROUP * BLOCKS
        load = max(deg[:HALF].sum(), deg[HALF:].sum()) / n_bins
        SUBT = max(1, int(math.ceil(load / 128)))
        for _try in range(3):
            cap = SUBT * 128
            try:
                bins_a = _pack_half(deg[:HALF], n_bins, cap)
                bins_b = _pack_half(deg[HALF:], n_bins, cap)
            except AssertionError:
                SUBT += 1
                continue
            if best is None or BLOCKS * SUBT < best[0] * best[1]:
                best = (BLOCKS, SUBT, bins_a, bins_b)
            break
    if best is None:
        raise RuntimeError("packing failed")
    BLOCKS, SUBT, bins_a, bins_b = best
    n_bins = GROUP * BLOCKS
    SLOTS = BLOCKS * 128
    T = BLOCKS * SUBT  # subtiles per core

    # ---- node -> (core, block, slot); core 0..GROUP-1 own half A
    core_of = np.full(N, -1, np.int32)
    block_of = np.full(N, -1, np.int32)
    slot_of = np.full(N, -1, np.int32)
    node_of = np.full((n_cores, SLOTS), -1, np.int64)
    for half, bins in ((0, bins_a), (1, bins_b)):
        for i, bl in enumerate(bins):
            c = half * GROUP + i % GROUP
            b = i // GROUP
            for s, nd_local in enumerate(sorted(bl)):
                nd = nd_local + half * HALF
                core_of[nd] = c
                block_of[nd] = b
                slot_of[nd] = s
                node_of[c, b * 128 + s] = nd
    assert (core_of >= 0).all()

    # opposite-half row of a node (position in the group AllGather output):
    # contribution order within each group AG is group position 0..GROUP-1.
    H1 = SLOTS
    own_row = block_of * 128 + slot_of
    gp = (core_of % GROUP).astype(np.int64)
    opp_row = gp * SLOTS + own_row

    # ---- per-core edge slot assignment
    # gather batch = BPB whole blocks (~2048 edge slots)
    BPB = max(1, 2048 // (SUBT * 128))
    GB = BPB * SUBT * 128  # edge slots per gather batch
    NB = (BLOCKS + BPB - 1) // BPB
    per_core = []
    ecore = core_of[dst]
    for c in range(n_cores):
        m = ecore == c
        es, ed = src[m], dst[m]
        eb = block_of[ed]
        order = np.argsort(eb, kind="stable")
        es, ed, eb = es[order], ed[order], eb[order]
        counts = np.bincount(eb, minlength=BLOCKS)
        assert counts.max() <= SUBT * 128
        kp_idx = np.zeros(T * 128, np.int16)
        qp_idx = np.zeros(T * 128, np.int16)
        dstrel = np.full(T * 128, -1.0, np.float32)
        pos = 0
        for b in range(BLOCKS):
            n_b = counts[b]
            sl = slice(b * SUBT * 128, b * SUBT * 128 + n_b)
            seg = slice(pos, pos + n_b)
            kp_idx[sl] = opp_row[es[seg]].astype(np.int16)
            # qp index is LOCAL to the gather batch's block range so the
            # per-batch gather only depends on those blocks' table writes
            g = b // BPB
            qp_idx[sl] = (own_row[ed[seg]] - g * BPB * 128).astype(np.int16)
            dstrel[sl] = slot_of[ed[seg]].astype(np.float32)
            pos += n_b
        assert opp_row[es].max() < 32768 and GROUP * SLOTS < 32768
        assert qp_idx.min() >= 0 and qp_idx.max() < BPB * 128

        # wrap indices: unwrapped[i] = arr[i%16, i//16], replicated to 128 parts
        def wrap(a):
            out = np.zeros((NB, 128, GB // 16), np.int16)
            for g in range(NB):
                seg = a[g * GB : (g + 1) * GB]
                w = np.zeros((16, GB // 16), np.int16)
                n = len(seg)
                ii = np.arange(n)
                w[ii % 16, ii // 16] = seg
                out[g] = np.tile(w, (8, 1))
            return out

        xo = np.zeros((SLOTS, EMB), np.float32)
        vmask = node_of[c] >= 0
        xo[vmask] = x[node_of[c][vmask]]
        p = (c + GROUP) % n_cores  # partner core (opposite half)
        xp = np.zeros((SLOTS, EMB), np.float32)
        vmaskp = node_of[p] >= 0
        xp[vmaskp] = x[node_of[p][vmaskp]]

        def xT(a):  # [SLOTS, EMB] -> [128, EMB//128, SLOTS] fp8
            return np.ascontiguousarray(
                a.reshape(SLOTS, EMB // 128, 128).transpose(2, 1, 0)
            ).astype(FP8)

        per_core.append(
            dict(
                xTp=xT(xp),
                xTo=xT(xo),
                x_own=np.ascontiguousarray(
                    xo.reshape(BLOCKS, 128, EMB).transpose(1, 0, 2)
                ).astype(BF16),
                kp_ix=wrap(kp_idx),
                qp_ix=wrap(qp_idx),
                dstrel=np.ascontiguousarray(dstrel.reshape(T, 128).T),
            )
        )

    # ---- folded weights
    W1k, W1q = W1[:hd].astype(np.float64), W1[hd:].astype(np.float64)
    bd = lambda A: np.block(
        [[A, np.zeros_like(A)], [np.zeros_like(A), A]]
    )  # [256,256]
    Wq = Wkqv[:, :EMB].astype(np.float64)
    Wk = Wkqv[:, EMB : 2 * EMB].astype(np.float64) / math.sqrt(hd)
    Wv = Wkqv[:, 2 * EMB :].astype(np.float64)
    Wkp = (Wk @ bd(W1k)) * SK
    Wqp = (Wq @ bd(W1q)) * SK
    Wkpq = Wkp + Wqp
    Wvs = Wv * SV

    if not (
        np.all(bkqv == 0) and np.all(b1 == 0) and np.all(bout == 0)
    ):
        raise NotImplementedError("nonzero bkqv/b1/bout not supported")

    # SBUF layout [128, 2, F]: [p, c, e] = W[c*128+p, e]
    def chunk(W, F):
        return np.ascontiguousarray(
            W.astype(np.float32).reshape(2, 128, F).transpose(1, 0, 2)
        ).astype(BF16)

    Wkv = np.concatenate([Wkp, Wvs], axis=1)   # [256, 512] -> kp | v
    Wqv = np.concatenate([Wqp, Wvs], axis=1)   # [256, 512] -> qp | vown
    W2s = (W2[:hd].astype(np.float64) / SK).astype(np.float32)  # [128, 1]
    CW = GB // 128  # subtile-columns per full gather batch
    w2row = np.tile(W2s.reshape(-1), 2)  # [256]: W2/SK for both heads
    w2rep = np.ascontiguousarray(
        np.broadcast_to(w2row, (128, CW, 256))
    ).astype(BF16)
    consts = dict(
        Wkv_c=chunk(Wkv, 512),
        Wqv_c=chunk(Wqv, 512),
        Wkpq_c=chunk(Wkpq, EMB),
        Wout_c=chunk(Wout.astype(np.float64), EMB),
        W2_c=np.ascontiguousarray(W2s.astype(BF16)),  # [128,1]
        W2rep_c=w2rep,
        iota_bc=np.tile(np.arange(128, dtype=BF16)[None, :], (128, 1)),
        iota_col=np.arange(128, dtype=np.float32)[:, None],
        ones_col=np.full((128, 1), SV, BF16),
    )
    meta = dict(
        N=N,
        EMB=EMB,
        HALF=HALF,
        GROUP=GROUP,
        BLOCKS=BLOCKS,
        SLOTS=SLOTS,
        SUBT=SUBT,
        T=T,
        GB=GB,
        NB=NB,
        BPB=BPB,
        CW=CW,
        H1=H1,
        n_cores=n_cores,
        b2=float(np.asarray(b2).reshape(-1)[0]),
    )
    return meta, per_core, consts, node_of


# ------------------------------------------------------------- bass program


def build_program(meta, repeats=1):
    import concourse.bass as bass
    import concourse.tile as tile
    from concourse import bacc, mybir

    AF = mybir.ActivationFunctionType
    ALU = mybir.AluOpType
    BF = mybir.dt.bfloat16
    F32 = mybir.dt.float32
    F8 = mybir.dt.float8e4
    I16 = mybir.dt.int16

    EMB = meta["EMB"]
    BLOCKS, SLOTS, SUBT, T = (
        meta["BLOCKS"],
        meta["SLOTS"],
        meta["SUBT"],
        meta["T"],
    )
    GB, NB, GROUP, BPB, CW = (
        meta["GB"], meta["NB"], meta["GROUP"], meta["BPB"], meta["CW"],
    )
    H1 = meta["H1"]
    n_cores = meta["n_cores"]
    b2f = meta["b2"]
    NB4 = (BLOCKS + 3) // 4  # phase-A 4-block chunks
    AG1_C4 = H1 // 512  # pass-1 chunks contributing to AllGather chunk 1

    nc = bacc.Bacc(
        "TRN2", target_bir_lowering=False, debug=False, num_devices=n_cores
    )

    dram = lambda name, shape, dt: nc.dram_tensor(
        name, shape, dt, kind="ExternalInput"
    )
    xTp_d = dram("xTp", [128, 2, SLOTS], F8)
    xTo_d = dram("xTo", [128, 2, SLOTS], F8)
    x_own_d = dram("x_own", [128, BLOCKS, EMB], BF)
    kp_ix_d = dram("kp_ix", [NB, 128, GB // 16], I16)
    qp_ix_d = dram("qp_ix", [NB, 128, GB // 16], I16)
    dstrel_d = dram("dstrel", [128, T], F32)
    Wkv_d = dram("Wkv_c", [128, 2, 512], BF)
    Wqv_d = dram("Wqv_c", [128, 2, 512], BF)
    Wkpq_d = dram("Wkpq_c", [128, 2, EMB], BF)
    Wout_d = dram("Wout_c", [128, 2, EMB], BF)
    W2_d = dram("W2_c", [128, 1], BF)
    W2rep_d = dram("W2rep_c", [128, CW, 256], BF)
    iota_bc_d = dram("iota_bc", [128, 128], BF)
    iota_col_d = dram("iota_col", [128, 1], F32)
    ones_col_d = dram("ones_col", [128, 1], BF)
    out_d = nc.dram_tensor("out", [128, BLOCKS, EMB], BF, kind="ExternalOutput")

    groups2 = [
        list(range(GROUP)),
        list(range(GROUP, n_cores)),
    ]

    with tile.TileContext(nc) as tc:
        # internal DRAM (collective buffers + local qp table)
        _frees = []

        def _dram_tile(shape, name, addr_space="Local"):
            t, free = tc.tile(
                shape,
                F8,
                space=bass.MemorySpace.DRAM,
                addr_space=addr_space,
                name=name,
            )
            _frees.append(free)
            return t

        kpv_agin = _dram_tile([SLOTS, 512], "kpv_agin")
        qp_tbl = _dram_tile([SLOTS, EMB], "qp_tbl")
        kpv_opp = _dram_tile([GROUP * SLOTS, 512], "kpv_opp", "Shared")

        with tc.tile_pool(name="res", bufs=1) as res:
            ST_res = res.tile([128, 2, SLOTS], BF)
            Vown_res = res.tile([128, BLOCKS, EMB], F8)
            x_res = res.tile([128, BLOCKS, EMB], BF)
            exS_res = res.tile([128, BLOCKS, 2], F32)
            dstrel_sb = res.tile([128, T], F32)
            iota_bc = res.tile([128, 128], BF)
            iota_col = res.tile([128, 1], F32)
            ones_col = res.tile([128, 1], BF)
            W2_sb = res.tile([128, 1], BF)
            W2rep_sb = res.tile([128, CW, 256], BF)
            Wkv_sb = res.tile([128, 2, 512], BF)
            Wqv_sb = res.tile([128, 2, 512], BF)
            Wkpq_sb = res.tile([128, 2, EMB], BF)
            Wout_sb = res.tile([128, 2, EMB], BF)

            def _body():
                nc.sync.dma_start(dstrel_sb[:], dstrel_d[:])
                nc.sync.dma_start(iota_bc[:], iota_bc_d[:])
                nc.sync.dma_start(iota_col[:], iota_col_d[:])
                nc.sync.dma_start(ones_col[:], ones_col_d[:])
                nc.sync.dma_start(W2_sb[:], W2_d[:])
                nc.sync.dma_start(W2rep_sb[:], W2rep_d[:])
                nc.sync.dma_start(x_res[:], x_own_d[:])
                for w_sb, w_d in (
                    (Wkv_sb, Wkv_d),
                    (Wqv_sb, Wqv_d),
                    (Wkpq_sb, Wkpq_d),
                    (Wout_sb, Wout_d),
                ):
                    nc.sync.dma_start(w_sb[:], w_d[:])

                # ---------- phase A pass 1: partner Kp|V table -> AllGather
                with (
                    tc.tile_pool(name="pA", bufs=3) as pA,
                    tc.tile_pool(name="psA", bufs=2, space="PSUM") as psA,
                ):
                    for c4 in range(NB4):
                        nb4 = min(BLOCKS - c4 * 4, 4)
                        w = nb4 * 128
                        sl = slice(c4 * 512, c4 * 512 + w)
                        xtp = pA.tile([128, 2, 512], F8, tag="xtp")
                        nc.sync.dma_start(xtp[:, :, :w], xTp_d[:, :, sl])
                        st1 = pA.tile([128, 4, 512], F8, tag="st1")
                        for bi in range(nb4):
                            bsl = slice(bi * 128, (bi + 1) * 128)
                            ps1 = psA.tile([128, 512], F32, tag="ps1")
                            nc.tensor.matmul(
                                ps1[:], xtp[:, 0, bsl], Wkv_sb[:, 0, :],
                                start=True, stop=False,
                            )
                            nc.tensor.matmul(
                                ps1[:], xtp[:, 1, bsl], Wkv_sb[:, 1, :],
                                start=False, stop=True,
                            )
                            nc.scalar.activation(st1[:, bi, :], ps1[:], AF.Copy)
                        nc.sync.dma_start(
                            kpv_agin[sl, :].rearrange("(b p) f -> p b f", p=128),
                            st1[:, :nb4, :],
                        )
                if os.environ.get("ABLATE_COLL"):
                    # timing ablation: local copy instead of the AllGather
                    # (numerics for 3/4 of the table are garbage)
                    nc.sync.dma_start(kpv_opp[0:SLOTS, :], kpv_agin[:])
                else:
                    nc.gpsimd.collective_compute(
                        "AllGather",
                        ALU.bypass,
                        replica_groups=groups2,
                        ins=[kpv_agin[:]],
                        outs=[kpv_opp[:]],
                    )

                # ---------- phase A pass 2 (overlaps collective):
                # own Qp table, Vown, ST; then self-loop logits
                with (
                    tc.tile_pool(name="pB", bufs=3) as pB,
                    tc.tile_pool(name="psB", bufs=2, space="PSUM") as psB,
                    tc.tile_pool(name="psS", bufs=1, space="PSUM") as psS,
                ):
                    aself = psS.tile([128, BLOCKS, 2], F32)
                    for c4 in range(NB4):
                        nb4 = min(BLOCKS - c4 * 4, 4)
                        w = nb4 * 128
                        sl = slice(c4 * 512, c4 * 512 + w)
                        xto = pB.tile([128, 2, 512], F8, tag="xto")
                        nc.sync.dma_start(xto[:, :, :w], xTo_d[:, :, sl])
                        st2 = pB.tile([128, 4, 256], F8, tag="st2")
                        for bi in range(nb4):
                            b = c4 * 4 + bi
                            bsl = slice(bi * 128, (bi + 1) * 128)
                            gsl = slice(b * 128, (b + 1) * 128)
                            ps2 = psB.tile([128, 512], F32, tag="ps2")
                            nc.tensor.matmul(
                                ps2[:], xto[:, 0, bsl], Wqv_sb[:, 0, :],
                                start=True, stop=False,
                            )
                            nc.tensor.matmul(
                                ps2[:], xto[:, 1, bsl], Wqv_sb[:, 1, :],
                                start=False, stop=True,
                            )
                            ps3 = psB.tile([128, 2, 128], F32, tag="ps3")
                            for c in range(2):
                                for f in range(2):
                                    nc.tensor.matmul(
                                        ps3[:, f, :],
                                        Wkpq_sb[:, c, f * 128 : (f + 1) * 128],
                                        xto[:, c, bsl],
                                        start=(c == 0 and f == 0),
                                        stop=(c == 1 and f == 1),
                                    )
                            nc.scalar.activation(st2[:, bi, :], ps2[:, 0:256], AF.Copy)
                            nc.scalar.activation(
                                Vown_res[:, b, :], ps2[:, 256:512], AF.Copy
                            )
                            nc.scalar.activation(
                                ST_res[:, :, gsl], ps3[:], AF.Copy
                            )
                            # self-loop logits for this block
                            rst = pB.tile([128, 2, 128], BF, tag="rst")
                            nc.vector.tensor_scalar_max(
                                rst[:], ST_res[:, :, gsl], 0.0
                            )
                            for h in range(2):
                                nc.tensor.matmul(
                                    aself[:, b, h : h + 1],
                                    rst[:, h, :],
                                    W2_sb[:],
                                    start=True,
                                    stop=True,
                                )
                        nc.sync.dma_start(
                            qp_tbl[sl, :].rearrange("(b p) f -> p b f", p=128),
                            st2[:, :nb4, :],
                        )
                    nc.scalar.activation(exS_res[:], aself[:], AF.Exp, bias=b2f)

                # ---------------- edge phase ----------------
                with (
                    tc.tile_pool(name="pG", bufs=2) as pG,
                    tc.tile_pool(name="pW", bufs=2) as pW,
                    tc.tile_pool(name="pE", bufs=3) as pE,
                    tc.tile_pool(name="psE", bufs=2, space="PSUM") as psE,
                ):
                    for g in range(NB):
                        nblk = min(BLOCKS - g * BPB, BPB)
                        C = nblk * SUBT
                        gb = C * 128
                        gb16 = gb // 16
                        kpix = pG.tile([128, GB // 16], I16, tag="kpix")
                        nc.sync.dma_start(kpix[:, :gb16], kp_ix_d[g, :, :gb16])
                        qpix = pG.tile([128, GB // 16], I16, tag="qpix")
                        nc.sync.dma_start(qpix[:, :gb16], qp_ix_d[g, :, :gb16])
                        KpV = pG.tile([128, GB // 128, 512], F8, tag="KpV")
                        nc.gpsimd.dma_gather(
                            KpV[:, :C, :],
                            kpv_opp[:],
                            kpix[:, :gb16],
                            num_idxs=gb,
                            num_idxs_reg=gb,
                            elem_size=512,
                            transpose=False,
                            single_packet=False,
                        )
                        Qp = pG.tile([128, GB // 128, 256], F8, tag="Qp")
                        nc.gpsimd.dma_gather(
                            Qp[:, :C, :],
                            qp_tbl[g * BPB * 128 : g * BPB * 128 + nblk * 128, :],
                            qpix[:, :gb16],
                            num_idxs=gb,
                            num_idxs_reg=gb,
                            elem_size=256,
                            transpose=False,
                            single_packet=False,
                        )
                        z = pW.tile([128, GB // 128, 256], BF, tag="z")
                        nc.vector.tensor_tensor(
                            z[:, :C, :], KpV[:, :C, 0:256], Qp[:, :C, :], ALU.add
                        )
                        rz = pW.tile([128, GB // 128, 256], BF, tag="rz")
                        nc.scalar.activation(rz[:, :C, :], z[:, :C, :], AF.Relu)
                        prod = pW.tile([128, GB // 128, 256], BF, tag="prod")
                        nc.vector.tensor_tensor(
                            prod[:, :C, :], rz[:, :C, :], W2rep_sb[:, :C, :],
                            ALU.mult,
                        )
                        ared = pE.tile([128, GB // 128, 2], F32, tag="ared")
                        nc.vector.tensor_reduce(
                            ared[:, :C, :],
                            prod[:, :C, :].rearrange(
                                "p c (h j) -> p c h j", h=2
                            ),
                            axis=mybir.AxisListType.X,
                            op=ALU.add,
                        )
                        ex = pE.tile([128, GB // 128, 2], F32, tag="ex")
                        nc.scalar.activation(
                            ex[:, :C, :], ared[:, :C, :], AF.Exp, bias=b2f
                        )

                        for j in range(nblk):
                            b = g * BPB + j
                            aggs = psE.tile([128, EMB + 2], F32, tag="aggs")
                            first = True
                            for s in range(SUBT):
                                cc = j * SUBT + s
                                t_idx = b * SUBT + s
                                for h in range(2):
                                    oh = pE.tile([128, 128], BF, tag="oh")
                                    nc.vector.tensor_scalar(
                                        oh[:],
                                        iota_bc[:],
                                        dstrel_sb[:, t_idx : t_idx + 1],
                                        ex[:, cc, h : h + 1],
                                        op0=ALU.is_equal,
                                        op1=ALU.mult,
                                    )
                                    nc.tensor.matmul(
                                        aggs[:, h * 128 : (h + 1) * 128],
                                        KpV[:, cc, 256 + h * 128 : 256 + (h + 1) * 128],
                                        oh[:],
                                        start=first,
                                        stop=False,
                                    )
                                    first = False
                                    nc.tensor.matmul(
                                        aggs[:, EMB + h : EMB + h + 1],
                                        oh[:],
                                        ones_col[:],
                                        start=False,
                                        stop=False,
                                    )
                            for h in range(2):
                                dg = pE.tile([128, 128], BF, tag="dg")
                                nc.vector.tensor_scalar(
                                    dg[:],
                                    iota_bc[:],
                                    iota_col[:],
                                    exS_res[:, b, h : h + 1],
                                    op0=ALU.is_equal,
                                    op1=ALU.mult,
                                )
                                nc.tensor.matmul(
                                    aggs[:, h * 128 : (h + 1) * 128],
                                    Vown_res[:, b, h * 128 : (h + 1) * 128],
                                    dg[:],
                                    start=False,
                                    stop=False,
                                )
                                nc.tensor.matmul(
                                    aggs[:, EMB + h : EMB + h + 1],
                                    dg[:],
                                    ones_col[:],
                                    start=False,
                                    stop=(h == 1),
                                )
                            r = pE.tile([128, 2], F32, tag="r")
                            nc.vector.reciprocal(r[:], aggs[:, EMB : EMB + 2])
                            ag0 = pE.tile([128, 128], BF, tag="ag0")
                            nc.vector.tensor_copy(ag0[:], aggs[:, 0:128])
                            ag1 = pE.tile([128, 128], BF, tag="ag1")
                            nc.vector.tensor_copy(ag1[:], aggs[:, 128:256])
                            P = psE.tile([128, 2, EMB], F32, tag="P")
                            nc.tensor.matmul(
                                P[:, 0, :], ag0[:], Wout_sb[:, 0, :],
                                start=True, stop=True,
                            )
                            nc.tensor.matmul(
                                P[:, 1, :], ag1[:], Wout_sb[:, 1, :],
                                start=True, stop=True,
                            )
                            t0 = pE.tile([128, EMB], BF, tag="t0")
                            nc.scalar.activation(
                                t0[:], P[:, 0, :], AF.Copy, scale=r[:, 0:1]
                            )
                            t1 = pE.tile([128, EMB], BF, tag="t1")
                            nc.scalar.activation(
                                t1[:], P[:, 1, :], AF.Copy, scale=r[:, 1:2]
                            )
                            u = pE.tile([128, EMB], BF, tag="u")
                            nc.vector.tensor_tensor(u[:], t0[:], t1[:], ALU.add)
                            rl = pE.tile([128, EMB], BF, tag="rl")
                            nc.scalar.activation(rl[:], u[:], AF.Relu)
                            if j == 0:
                                ost = pE.tile(
                                    [128, BPB, EMB], BF, tag="ost", name="ost"
                                )
                            nc.vector.tensor_tensor(
                                ost[:, j, :], rl[:], x_res[:, b, :], ALU.add
                            )
                        nc.sync.dma_start(
                            out_d[:, g * BPB : g * BPB + nblk, :],
                            ost[:, :nblk, :],
                        )

            for _rep in range(repeats):
                _body()

        for f in _frees:
            f()

    nc.compile()
    return nc


# ------------------------------------------------------------------ driver


def assemble_out(oc, node_of_c, N, EMB, out):
    """Scatter one core's [128, BLOCKS, EMB] output into the full array."""
    blocks = oc.shape[1]
    flat = np.asarray(oc).astype(np.float32).transpose(1, 0, 2).reshape(-1, EMB)
    valid = node_of_c >= 0
    out[node_of_c[valid]] = flat[valid]


def _build_all(inputs, n_cores=8, repeats=1):
    meta, per_core, consts, node_of = prep(n_cores=n_cores, **inputs)
    nc = build_program(meta, repeats=repeats)
    in_maps = []
    for c in range(n_cores):
        m = dict(per_core[c])
        m.update(consts)
        in_maps.append({k: np.ascontiguousarray(v) for k, v in m.items()})
    return meta, nc, in_maps, node_of


def kernel(**inputs):
    import concourse.bass_utils as bass_utils

    n_cores = 8
    meta, nc, in_maps, node_of = _build_all(inputs, n_cores)
    res = bass_utils.run_bass_kernel_spmd(
        nc, in_maps, core_ids=list(range(n_cores))
    )
    N, EMB = inputs["x"].shape
    out = np.zeros((N, EMB), np.float32)
    for c in range(n_cores):
        assemble_out(res.results[c]["out"], node_of[c], N, EMB, out)
    return out


# revision 27
# speedup vs baseline: 1.1952x; 1.0332x over previous
"""Trainium2 Bass kernel for MetaGNNNoEdgeAttr (GNN message passing).

Strategy (8 NeuronCores, SPMD):
  - Undirected+self-loop graph; bipartite => edges into half-A nodes have
    src in half B and vice versa.
  - Nodes are packed into (core, block, slot): 4 cores per half, BLOCKS
    blocks of 128 slots per core, degree-balanced so every block has at
    most SUBT*128 incoming directed edges -> fully uniform SPMD schedule.
  - Algebra: kqv/W1 folded on host:  Kp = x @ Wkp, Qp = x @ Wqp, V = x @ Wv
    with Wkp = (Wk/sqrt(hd)) @ blockdiag(W1k) etc.  Then per edge
    a = W2^T relu(Kp[src] + Qp[dst]) and out_i = (sum_e exp(a) * V[src]) /
    (sum_e exp(a)) @ Wout (+x, relu).  Segment max is skipped (|a| ~ 0.07).
  - fp8 tables: Kp/Qp scaled by SK=256 (relu is pos-homogeneous, W2/SK
    absorbs it), V scaled by SV=8 (ones_col=SV makes the softmax ratio
    exact).  Kp and V fused in ONE [SLOTS, 512] fp8 table -> a single fp8
    AllGather (per half-group) and a single 512B-per-edge gather.
  - Phase A pass 1 (per core): Kp|V for partner core's nodes -> fp8 table
    -> group AllGather.  Pass 2 (overlaps the collective): Qp table (fp8,
    local), Vown (fp8 resident), ST=(Kp+Qp)^T (bf16 resident) for own
    nodes; then self-loop logits a_self = W2'.relu(ST) for all blocks.
  - Edge phase per gather batch: one fused KpV gather (edge-major fp8) +
    one Qp gather; z=Kp+Qp (DVE), relu (ACT), a = rowsum(rz*W2rep) (DVE
    mult + per-head strided reduce); exp on ACT; per (subtile, head)
    one-hot(dst)*exp via fused tensor_scalar; aggregation + softmax
    denominator via PE matmuls into PSUM; self-loops via resident
    Vown/exp(a_self) diagonals; Wout per head; per-node normalization via
    ACT scale; relu; +x residual (bf16); batched output stores.
"""

import os
import sys
import math
import numpy as np

for _p in ("/opt/trn_rl_repo", "/root/.axon_site/_ro/trn_rl_repo"):
    if os.path.isdir(_p) and _p not in sys.path:
        sys.path.insert(0, _p)

import ml_dtypes  # noqa: E402

BF16 = ml_dtypes.bfloat16
FP8 = ml_dtypes.float8_e4m3

SK = 256.0  # Kp/Qp fp8 scale (folded into Wkp/Wqp; W2 divided by it)
SV = 8.0    # V fp8 scale (ones_col = SV cancels it in the softmax ratio)

# ---------------------------------------------------------------- host prep


def _pack_half(deg_half, n_bins, cap_edges):
    """Pack nodes (by local id within half) into n_bins bins of <=128 nodes,
    each bin with sum(deg) <= cap_edges. Returns [n_bins] lists of local ids."""
    nh = deg_half.shape[0]
    order = np.argsort(-deg_half, kind="stable")
    per_bin = (nh + n_bins - 1) // n_bins
    assert per_bin <= 128
    bins = [[] for _ in range(n_bins)]
    loads = np.zeros(n_bins, dtype=np.int64)
    # deal degree-sorted nodes in rounds: heaviest of each round to the
    # currently lightest bins
    for r in range(per_bin):
        chunk = order[r * n_bins : (r + 1) * n_bins]
        target = np.argsort(loads, kind="stable")
        for k, nd in enumerate(chunk):
            b = int(target[k])
            bins[b].append(int(nd))
            loads[b] += deg_half[nd]
    # fixup: move low-degree nodes from overloaded to underloaded bins
    sizes = np.array([len(x) for x in bins])
    for _ in range(20000):
        hi = int(np.argmax(loads))
        if loads[hi] <= cap_edges:
            break
        lo = int(np.argmin(loads + (sizes >= 128) * 10**9))
        bl = bins[hi]
        j = int(np.argmin(deg_half[bl]))
        nd = bl.pop(j)
        bins[lo].append(nd)
        loads[hi] -= deg_half[nd]
        loads[lo] += deg_half[nd]
        sizes[hi] -= 1
        sizes[lo] += 1
    assert loads.max() <= cap_edges, (loads.max(), cap_edges)
    assert max(len(b) for b in bins) <= 128
    return bins


def prep(x, edge_index, Wkqv, bkqv, W1, b1, W2, b2, Wout, bout, n_cores=8):
    """All host-side preprocessing. Returns (meta, per_core_inputs, const_inputs,
    scatter info for output assembly)."""
    N, EMB = x.shape
    H, hd = 2, EMB // 2
    HALF = N // 2
    GROUP = n_cores // 2

    ei = np.asarray(edge_index).astype(np.int64)
    src = np.concatenate([ei[0], ei[1]])
    dst = np.concatenate([ei[1], ei[0]])
    assert src.min() >= 0 and src.max() < N

    deg = np.bincount(dst, minlength=N)

    # ---- choose BLOCKS / SUBT: minimize total subtiles T = BLOCKS * SUBT
    B0 = max(1, (HALF + GROUP * 128 - 1) // (GROUP * 128))
    best = None
    for BLOCKS in range(B0, B0 + 4):
        n_bins = GROUP * BLOCKS
        load = max(deg[:HALF].sum(), deg[HALF:].sum()) / n_bins
        SUBT = max(1, int(math.ceil(load / 128)))
        for _try in range(3):
            cap = SUBT * 128
            try:
                bins_a = _pack_half(deg[:HALF], n_bins, cap)
                bins_b = _pack_half(deg[HALF:], n_bins, cap)
            except AssertionError:
                SUBT += 1
                continue
            if best is None or BLOCKS * SUBT < best[0] * best[1]:
                best = (BLOCKS, SUBT, bins_a, bins_b)
            break
    if best is None:
        raise RuntimeError("packing failed")
    BLOCKS, SUBT, bins_a, bins_b = best
    n_bins = GROUP * BLOCKS
    SLOTS = BLOCKS * 128
    T = BLOCKS * SUBT  # subtiles per core

    # ---- node -> (core, block, slot); core 0..GROUP-1 own half A
    core_of = np.full(N, -1, np.int32)
    block_of = np.full(N, -1, np.int32)
    slot_of = np.full(N, -1, np.int32)
    node_of = np.full((n_cores, SLOTS), -1, np.int64)
    for half, bins in ((0, bins_a), (1, bins_b)):
        for i, bl in enumerate(bins):
            c = half * GROUP + i % GROUP
            b = i // GROUP
            for s, nd_local in enumerate(sorted(bl)):
                nd = nd_local + half * HALF
                core_of[nd] = c
                block_of[nd] = b
                slot_of[nd] = s
                node_of[c, b * 128 + s] = nd
    assert (core_of >= 0).all()

    # opposite-half row of a node (position in the group AllGather output):
    # contribution order within each group AG is group position 0..GROUP-1.
    H1 = SLOTS
    own_row = block_of * 128 + slot_of
    gp = (core_of % GROUP).astype(np.int64)
    opp_row = gp * SLOTS + own_row

    # ---- per-core edge slot assignment
    # gather batch = BPB whole blocks (~2048 edge slots)
    BPB = max(1, 2048 // (SUBT * 128))
    GB = BPB * SUBT * 128  # edge slots per gather batch
    NB = (BLOCKS + BPB - 1) // BPB
    per_core = []
    ecore = core_of[dst]
    for c in range(n_cores):
        m = ecore == c
        es, ed = src[m], dst[m]
        eb = block_of[ed]
        order = np.argsort(eb, kind="stable")
        es, ed, eb = es[order], ed[order], eb[order]
        counts = np.bincount(eb, minlength=BLOCKS)
        assert counts.max() <= SUBT * 128
        kp_idx = np.zeros(T * 128, np.int16)
        qp_idx = np.zeros(T * 128, np.int16)
        dstrel = np.full(T * 128, -1.0, np.float32)
        pos = 0
        for b in range(BLOCKS):
            n_b = counts[b]
            sl = slice(b * SUBT * 128, b * SUBT * 128 + n_b)
            seg = slice(pos, pos + n_b)
            kp_idx[sl] = opp_row[es[seg]].astype(np.int16)
            # qp index is LOCAL to the gather batch's block range so the
            # per-batch gather only depends on those blocks' table writes
            g = b // BPB
            qp_idx[sl] = (own_row[ed[seg]] - g * BPB * 128).astype(np.int16)
            dstrel[sl] = slot_of[ed[seg]].astype(np.float32)
            pos += n_b
        assert opp_row[es].max() < 32768 and GROUP * SLOTS < 32768
        assert qp_idx.min() >= 0 and qp_idx.max() < BPB * 128

        # wrap indices: unwrapped[i] = arr[i%16, i//16], replicated to 128 parts
        def wrap(a):
            out = np.zeros((NB, 128, GB // 16), np.int16)
            for g in range(NB):
                seg = a[g * GB : (g + 1) * GB]
                w = np.zeros((16, GB // 16), np.int16)
                n = len(seg)
                ii = np.arange(n)
                w[ii % 16, ii // 16] = seg
                out[g] = np.tile(w, (8, 1))
            return out

        xo = np.zeros((SLOTS, EMB), np.float32)
        vmask = node_of[c] >= 0
        xo[vmask] = x[node_of[c][vmask]]
        p = (c + GROUP) % n_cores  # partner core (opposite half)
        xp = np.zeros((SLOTS, EMB), np.float32)
        vmaskp = node_of[p] >= 0
        xp[vmaskp] = x[node_of[p][vmaskp]]

        def xT(a):  # [SLOTS, EMB] -> [128, EMB//128, SLOTS] fp8
            return np.ascontiguousarray(
                a.reshape(SLOTS, EMB // 128, 128).transpose(2, 1, 0)
            ).astype(FP8)

        per_core.append(
            dict(
                xTp=xT(xp),
                xTo=xT(xo),
                x_own=np.ascontiguousarray(
                    xo.reshape(BLOCKS, 128, EMB).transpose(1, 0, 2)
                ).astype(BF16),
                kp_ix=wrap(kp_idx),
                qp_ix=wrap(qp_idx),
                dstrel=np.ascontiguousarray(dstrel.reshape(T, 128).T),
            )
        )

    # ---- folded weights
    W1k, W1q = W1[:hd].astype(np.float64), W1[hd:].astype(np.float64)
    bd = lambda A: np.block(
        [[A, np.zeros_like(A)], [np.zeros_like(A), A]]
    )  # [256,256]
    Wq = Wkqv[:, :EMB].astype(np.float64)
    Wk = Wkqv[:, EMB : 2 * EMB].astype(np.float64) / math.sqrt(hd)
    Wv = Wkqv[:, 2 * EMB :].astype(np.float64)
    Wkp = (Wk @ bd(W1k)) * SK
    Wqp = (Wq @ bd(W1q)) * SK
    Wkpq = Wkp + Wqp
    Wvs = Wv * SV

    if not (
        np.all(bkqv == 0) and np.all(b1 == 0) and np.all(bout == 0)
    ):
        raise NotImplementedError("nonzero bkqv/b1/bout not supported")

    # SBUF layout [128, 2, F]: [p, c, e] = W[c*128+p, e]
    def chunk(W, F):
        return np.ascontiguousarray(
            W.astype(np.float32).reshape(2, 128, F).transpose(1, 0, 2)
        ).astype(BF16)

    Wkv = np.concatenate([Wkp, Wvs], axis=1)   # [256, 512] -> kp | v
    Wqv = np.concatenate([Wqp, Wvs], axis=1)   # [256, 512] -> qp | vown
    W2s = (W2[:hd].astype(np.float64) / SK).astype(np.float32)  # [128, 1]
    CW = GB // 128  # subtile-columns per full gather batch
    w2row = np.tile(W2s.reshape(-1), 2)  # [256]: W2/SK for both heads
    w2rep = np.ascontiguousarray(
        np.broadcast_to(w2row, (128, CW, 256))
    ).astype(BF16)
    consts = dict(
        Wkv_c=chunk(Wkv, 512),
        Wqv_c=chunk(Wqv, 512),
        Wkpq_c=chunk(Wkpq, EMB),
        Wout_c=chunk(Wout.astype(np.float64), EMB),
        W2_c=np.ascontiguousarray(W2s.astype(BF16)),  # [128,1]
        W2rep_c=w2rep,
        iota_bc=np.tile(np.arange(128, dtype=BF16)[None, :], (128, 1)),
        iota_col=np.arange(128, dtype=np.float32)[:, None],
        ones_col=np.full((128, 1), SV, BF16),
    )
    meta = dict(
        N=N,
        EMB=EMB,
        HALF=HALF,
        GROUP=GROUP,
        BLOCKS=BLOCKS,
        SLOTS=SLOTS,
        SUBT=SUBT,
        T=T,
        GB=GB,
        NB=NB,
        BPB=BPB,
        CW=CW,
        H1=H1,
        n_cores=n_cores,
        b2=float(np.asarray(b2).reshape(-1)[0]),
    )
    return meta, per_core, consts, node_of


# ------------------------------------------------------------- bass program


def build_program(meta, repeats=1):
    import concourse.bass as bass
    import concourse.tile as tile
    from concourse import bacc, mybir

    AF = mybir.ActivationFunctionType
    ALU = mybir.AluOpType
    BF = mybir.dt.bfloat16
    F32 = mybir.dt.float32
    F8 = mybir.dt.float8e4
    I16 = mybir.dt.int16

    EMB = meta["EMB"]
    BLOCKS, SLOTS, SUBT, T = (
        meta["BLOCKS"],
        meta["SLOTS"],
        meta["SUBT"],
        meta["T"],
    )
    GB, NB, GROUP, BPB, CW = (
        meta["GB"], meta["NB"], meta["GROUP"], meta["BPB"], meta["CW"],
    )
    H1 = meta["H1"]
    n_cores = meta["n_cores"]
    b2f = meta["b2"]
    NB4 = (BLOCKS + 3) // 4  # phase-A 4-block chunks
    AG1_C4 = H1 // 512  # pass-1 chunks contributing to AllGather chunk 1

    nc = bacc.Bacc(
        "TRN2", target_bir_lowering=False, debug=False, num_devices=n_cores
    )

    dram = lambda name, shape, dt: nc.dram_tensor(
        name, shape, dt, kind="ExternalInput"
    )
    xTp_d = dram("xTp", [128, 2, SLOTS], F8)
    xTo_d = dram("xTo", [128, 2, SLOTS], F8)
    x_own_d = dram("x_own", [128, BLOCKS, EMB], BF)
    kp_ix_d = dram("kp_ix", [NB, 128, GB // 16], I16)
    qp_ix_d = dram("qp_ix", [NB, 128, GB // 16], I16)
    dstrel_d = dram("dstrel", [128, T], F32)
    Wkv_d = dram("Wkv_c", [128, 2, 512], BF)
    Wqv_d = dram("Wqv_c", [128, 2, 512], BF)
    Wkpq_d = dram("Wkpq_c", [128, 2, EMB], BF)
    Wout_d = dram("Wout_c", [128, 2, EMB], BF)
    W2_d = dram("W2_c", [128, 1], BF)
    W2rep_d = dram("W2rep_c", [128, CW, 256], BF)
    iota_bc_d = dram("iota_bc", [128, 128], BF)
    iota_col_d = dram("iota_col", [128, 1], F32)
    ones_col_d = dram("ones_col", [128, 1], BF)
    out_d = nc.dram_tensor("out", [128, BLOCKS, EMB], BF, kind="ExternalOutput")

    groups2 = [
        list(range(GROUP)),
        list(range(GROUP, n_cores)),
    ]

    with tile.TileContext(nc) as tc:
        # internal DRAM (collective buffers + local qp table)
        _frees = []

        def _dram_tile(shape, name, addr_space="Local"):
            t, free = tc.tile(
                shape,
                F8,
                space=bass.MemorySpace.DRAM,
                addr_space=addr_space,
                name=name,
            )
            _frees.append(free)
            return t

        kpv_agin = _dram_tile([SLOTS, 512], "kpv_agin")
        qp_tbl = _dram_tile([SLOTS, EMB], "qp_tbl")
        kpv_opp = _dram_tile([GROUP * SLOTS, 512], "kpv_opp", "Shared")

        with tc.tile_pool(name="res", bufs=1) as res:
            ST_res = res.tile([128, 2, SLOTS], BF)
            Vown_res = res.tile([128, BLOCKS, EMB], F8)
            x_res = res.tile([128, BLOCKS, EMB], BF)
            exS_res = res.tile([128, BLOCKS, 2], F32)
            dstrel_sb = res.tile([128, T], F32)
            iota_bc = res.tile([128, 128], BF)
            iota_col = res.tile([128, 1], F32)
            ones_col = res.tile([128, 1], BF)
            W2_sb = res.tile([128, 1], BF)
            W2rep_sb = res.tile([128, CW, 256], BF)
            Wkv_sb = res.tile([128, 2, 512], BF)
            Wqv_sb = res.tile([128, 2, 512], BF)
            Wkpq_sb = res.tile([128, 2, EMB], BF)
            Wout_sb = res.tile([128, 2, EMB], BF)

            def _body():
                nc.sync.dma_start(dstrel_sb[:], dstrel_d[:])
                nc.sync.dma_start(iota_bc[:], iota_bc_d[:])
                nc.sync.dma_start(iota_col[:], iota_col_d[:])
                nc.sync.dma_start(ones_col[:], ones_col_d[:])
                nc.sync.dma_start(W2_sb[:], W2_d[:])
                nc.sync.dma_start(W2rep_sb[:], W2rep_d[:])
                nc.sync.dma_start(x_res[:], x_own_d[:])
                for w_sb, w_d in (
                    (Wkv_sb, Wkv_d),
                    (Wqv_sb, Wqv_d),
                    (Wkpq_sb, Wkpq_d),
                    (Wout_sb, Wout_d),
                ):
                    nc.sync.dma_start(w_sb[:], w_d[:])

                # ---------- phase A pass 1: partner Kp|V table -> AllGather
                with (
                    tc.tile_pool(name="pA", bufs=3) as pA,
                    tc.tile_pool(name="psA", bufs=2, space="PSUM") as psA,
                ):
                    for c4 in range(NB4):
                        nb4 = min(BLOCKS - c4 * 4, 4)
                        w = nb4 * 128
                        sl = slice(c4 * 512, c4 * 512 + w)
                        xtp = pA.tile([128, 2, 512], F8, tag="xtp")
                        nc.sync.dma_start(xtp[:, :, :w], xTp_d[:, :, sl])
                        st1 = pA.tile([128, 4, 512], F8, tag="st1")
                        for bi in range(nb4):
                            bsl = slice(bi * 128, (bi + 1) * 128)
                            ps1 = psA.tile([128, 512], F32, tag="ps1")
                            nc.tensor.matmul(
                                ps1[:], xtp[:, 0, bsl], Wkv_sb[:, 0, :],
                                start=True, stop=False,
                            )
                            nc.tensor.matmul(
                                ps1[:], xtp[:, 1, bsl], Wkv_sb[:, 1, :],
                                start=False, stop=True,
                            )
                            nc.scalar.activation(st1[:, bi, :], ps1[:], AF.Copy)
                        nc.sync.dma_start(
                            kpv_agin[sl, :].rearrange("(b p) f -> p b f", p=128),
                            st1[:, :nb4, :],
                        )
                if os.environ.get("ABLATE_COLL"):
                    # timing ablation: local copy instead of the AllGather
                    # (numerics for 3/4 of the table are garbage)
                    nc.sync.dma_start(kpv_opp[0:SLOTS, :], kpv_agin[:])
                else:
                    nc.gpsimd.collective_compute(
                        "AllGather",
                        ALU.bypass,
                        replica_groups=groups2,
                        ins=[kpv_agin[:]],
                        outs=[kpv_opp[:]],
                    )

                # ---------- phase A pass 2 (overlaps collective):
                # own Qp table, Vown, ST; then self-loop logits
                with (
                    tc.tile_pool(name="pB", bufs=3) as pB,
                    tc.tile_pool(name="psB", bufs=2, space="PSUM") as psB,
                    tc.tile_pool(name="psS", bufs=1, space="PSUM") as psS,
                ):
                    aself = psS.tile([128, BLOCKS, 2], F32)
                    for c4 in range(NB4):
                        nb4 = min(BLOCKS - c4 * 4, 4)
                        w = nb4 * 128
                        sl = slice(c4 * 512, c4 * 512 + w)
                        xto = pB.tile([128, 2, 512], F8, tag="xto")
                        nc.sync.dma_start(xto[:, :, :w], xTo_d[:, :, sl])
                        st2 = pB.tile([128, 4, 256], F8, tag="st2")
                        for bi in range(nb4):
                            b = c4 * 4 + bi
                            bsl = slice(bi * 128, (bi + 1) * 128)
                            gsl = slice(b * 128, (b + 1) * 128)
                            ps2 = psB.tile([128, 512], F32, tag="ps2")
                            nc.tensor.matmul(
                                ps2[:], xto[:, 0, bsl], Wqv_sb[:, 0, :],
                                start=True, stop=False,
                            )
                            nc.tensor.matmul(
                                ps2[:], xto[:, 1, bsl], Wqv_sb[:, 1, :],
                                start=False, stop=True,
                            )
                            ps3 = psB.tile([128, 2, 128], F32, tag="ps3")
                            for c in range(2):
                                for f in range(2):
                                    nc.tensor.matmul(
                                        ps3[:, f, :],
                                        Wkpq_sb[:, c, f * 128 : (f + 1) * 128],
                                        xto[:, c, bsl],
                                        start=(c == 0 and f == 0),
                                        stop=(c == 1 and f == 1),
                                    )
                            nc.scalar.activation(st2[:, bi, :], ps2[:, 0:256], AF.Copy)
                            nc.scalar.activation(
                                Vown_res[:, b, :], ps2[:, 256:512], AF.Copy
                            )
                            nc.scalar.activation(
                                ST_res[:, :, gsl], ps3[:], AF.Copy
                            )
                            # self-loop logits for this block
                            rst = pB.tile([128, 2, 128], BF, tag="rst")
                            nc.vector.tensor_scalar_max(
                                rst[:], ST_res[:, :, gsl], 0.0
                            )
                            for h in range(2):
                                nc.tensor.matmul(
                                    aself[:, b, h : h + 1],
                                    rst[:, h, :],
                                    W2_sb[:],
                                    start=True,
                                    stop=True,
                                )
                        nc.sync.dma_start(
                            qp_tbl[sl, :].rearrange("(b p) f -> p b f", p=128),
                            st2[:, :nb4, :],
                        )
                    nc.scalar.activation(exS_res[:], aself[:], AF.Exp, bias=b2f)

                # ---------------- edge phase ----------------
                with (
                    tc.tile_pool(name="pG", bufs=3) as pG,
                    tc.tile_pool(name="pW", bufs=2) as pW,
                    tc.tile_pool(name="pE", bufs=3) as pE,
                    tc.tile_pool(name="psE", bufs=2, space="PSUM") as psE,
                ):
                    for g in range(NB):
                        nblk = min(BLOCKS - g * BPB, BPB)
                        C = nblk * SUBT
                        gb = C * 128
                        gb16 = gb // 16
                        kpix = pG.tile([128, GB // 16], I16, tag="kpix")
                        nc.sync.dma_start(kpix[:, :gb16], kp_ix_d[g, :, :gb16])
                        qpix = pG.tile([128, GB // 16], I16, tag="qpix")
                        nc.sync.dma_start(qpix[:, :gb16], qp_ix_d[g, :, :gb16])
                        KpV = pG.tile([128, GB // 128, 512], F8, tag="KpV")
                        nc.gpsimd.dma_gather(
                            KpV[:, :C, :],
                            kpv_opp[:],
                            kpix[:, :gb16],
                            num_idxs=gb,
                            num_idxs_reg=gb,
                            elem_size=512,
                            transpose=False,
                            single_packet=False,
                        )
                        Qp = pG.tile([128, GB // 128, 256], F8, tag="Qp")
                        nc.gpsimd.dma_gather(
                            Qp[:, :C, :],
                            qp_tbl[g * BPB * 128 : g * BPB * 128 + nblk * 128, :],
                            qpix[:, :gb16],
                            num_idxs=gb,
                            num_idxs_reg=gb,
                            elem_size=256,
                            transpose=False,
                            single_packet=False,
                        )
                        z = pW.tile([128, GB // 128, 256], BF, tag="z")
                        nc.vector.tensor_tensor(
                            z[:, :C, :], KpV[:, :C, 0:256], Qp[:, :C, :], ALU.add
                        )
                        rz = pW.tile([128, GB // 128, 256], BF, tag="rz")
                        nc.scalar.activation(rz[:, :C, :], z[:, :C, :], AF.Relu)
                        prod = pW.tile([128, GB // 128, 256], BF, tag="prod")
                        nc.vector.tensor_tensor(
                            prod[:, :C, :], rz[:, :C, :], W2rep_sb[:, :C, :],
                            ALU.mult,
                        )
                        ared = pE.tile([128, GB // 128, 2], F32, tag="ared")
                        nc.vector.tensor_reduce(
                            ared[:, :C, :],
                            prod[:, :C, :].rearrange(
                                "p c (h j) -> p c h j", h=2
                            ),
                            axis=mybir.AxisListType.X,
                            op=ALU.add,
                        )
                        ex = pE.tile([128, GB // 128, 2], F32, tag="ex")
                        nc.scalar.activation(
                            ex[:, :C, :], ared[:, :C, :], AF.Exp, bias=b2f
                        )

                        for j in range(nblk):
                            b = g * BPB + j
                            aggs = psE.tile([128, EMB + 2], F32, tag="aggs")
                            first = True
                            for s in range(SUBT):
                                cc = j * SUBT + s
                                t_idx = b * SUBT + s
                                for h in range(2):
                                    oh = pE.tile([128, 128], BF, tag="oh")
                                    nc.vector.tensor_scalar(
                                        oh[:],
                                        iota_bc[:],
                                        dstrel_sb[:, t_idx : t_idx + 1],
                                        ex[:, cc, h : h + 1],
                                        op0=ALU.is_equal,
                                        op1=ALU.mult,
                                    )
                                    nc.tensor.matmul(
                                        aggs[:, h * 128 : (h + 1) * 128],
                                        KpV[:, cc, 256 + h * 128 : 256 + (h + 1) * 128],
                                        oh[:],
                                        start=first,
                                        stop=False,
                                    )
                                    first = False
                                    nc.tensor.matmul(
                                        aggs[:, EMB + h : EMB + h + 1],
                                        oh[:],
                                        ones_col[:],
                                        start=False,
                                        stop=False,
                                    )
                            for h in range(2):
                                dg = pE.tile([128, 128], BF, tag="dg")
                                nc.vector.tensor_scalar(
                                    dg[:],
                                    iota_bc[:],
                                    iota_col[:],
                                    exS_res[:, b, h : h + 1],
                                    op0=ALU.is_equal,
                                    op1=ALU.mult,
                                )
                                nc.tensor.matmul(
                                    aggs[:, h * 128 : (h + 1) * 128],
                                    Vown_res[:, b, h * 128 : (h + 1) * 128],
                                    dg[:],
                                    start=False,
                                    stop=False,
                                )
                                nc.tensor.matmul(
                                    aggs[:, EMB + h : EMB + h + 1],
                                    dg[:],
                                    ones_col[:],
                                    start=False,
                                    stop=(h == 1),
                                )
                            r = pE.tile([128, 2], F32, tag="r")
                            nc.vector.reciprocal(r[:], aggs[:, EMB : EMB + 2])
                            ag0 = pE.tile([128, 128], BF, tag="ag0")
                            nc.scalar.activation(ag0[:], aggs[:, 0:128], AF.Copy)
                            ag1 = pE.tile([128, 128], BF, tag="ag1")
                            nc.scalar.activation(ag1[:], aggs[:, 128:256], AF.Copy)
                            P = psE.tile([128, 2, EMB], F32, tag="P")
                            nc.tensor.matmul(
                                P[:, 0, :], ag0[:], Wout_sb[:, 0, :],
                                start=True, stop=True,
                            )
                            nc.tensor.matmul(
                                P[:, 1, :], ag1[:], Wout_sb[:, 1, :],
                                start=True, stop=True,
                            )
                            t0 = pE.tile([128, EMB], BF, tag="t0")
                            nc.scalar.activation(
                                t0[:], P[:, 0, :], AF.Copy, scale=r[:, 0:1]
                            )
                            t1 = pE.tile([128, EMB], BF, tag="t1")
                            nc.scalar.activation(
                                t1[:], P[:, 1, :], AF.Copy, scale=r[:, 1:2]
                            )
                            u = pE.tile([128, EMB], BF, tag="u")
                            nc.vector.tensor_tensor(u[:], t0[:], t1[:], ALU.add)
                            rl = pE.tile([128, EMB], BF, tag="rl")
                            nc.scalar.activation(rl[:], u[:], AF.Relu)
                            if j == 0:
                                ost = pE.tile(
                                    [128, BPB, EMB], BF, tag="ost", name="ost"
                                )
                            nc.vector.tensor_tensor(
                                ost[:, j, :], rl[:], x_res[:, b, :], ALU.add
                            )
                        nc.sync.dma_start(
                            out_d[:, g * BPB : g * BPB + nblk, :],
                            ost[:, :nblk, :],
                        )

            for _rep in range(repeats):
                _body()

        for f in _frees:
            f()

    nc.compile()
    return nc


# ------------------------------------------------------------------ driver


def assemble_out(oc, node_of_c, N, EMB, out):
    """Scatter one core's [128, BLOCKS, EMB] output into the full array."""
    blocks = oc.shape[1]
    flat = np.asarray(oc).astype(np.float32).transpose(1, 0, 2).reshape(-1, EMB)
    valid = node_of_c >= 0
    out[node_of_c[valid]] = flat[valid]


def _build_all(inputs, n_cores=8, repeats=1):
    meta, per_core, consts, node_of = prep(n_cores=n_cores, **inputs)
    nc = build_program(meta, repeats=repeats)
    in_maps = []
    for c in range(n_cores):
        m = dict(per_core[c])
        m.update(consts)
        in_maps.append({k: np.ascontiguousarray(v) for k, v in m.items()})
    return meta, nc, in_maps, node_of


def kernel(**inputs):
    import concourse.bass_utils as bass_utils

    n_cores = 8
    meta, nc, in_maps, node_of = _build_all(inputs, n_cores)
    res = bass_utils.run_bass_kernel_spmd(
        nc, in_maps, core_ids=list(range(n_cores))
    )
    N, EMB = inputs["x"].shape
    out = np.zeros((N, EMB), np.float32)
    for c in range(n_cores):
        assemble_out(res.results[c]["out"], node_of[c], N, EMB, out)
    return out
